# revision 31
# baseline (speedup 1.0000x reference)
"""Trainium2 Bass kernel for nn_MultiHeadAttention_64158221467955.

Sliding-window (KER=3) patch multi-head attention:
    qp/kp/vp = patch_heads(x @ W + b)  -> [B,H,L,192]
    attn = softmax(qp @ kp^T / 24)     -> [B,H,L,L]   (returned)
    out  = combine(attn @ vp) @ wo + bo -> [B,L,512]  (returned)

Sharding: 8 cores = (batch b in {0,1}) x (head-pair p in {0..3}).
Each core computes its pair's projections (weight columns host-sliced),
full attention for its 2 heads, and a partial output projection
(wo rows host-sliced); host sums partials and adds bo.

The patch structure means head-pair p's 384 channels are columns
(384p..384p+384)%512 of x@W, each 128-chunk read at a uniform time shift
s in {0,1,2}.  The shift varies per core, so it is fed as *data* (an
int32 offsets tensor) and applied with dynamic (register-driven) access
pattern offsets, keeping a single SPMD program.
"""

import os
import time

# This kernel executes on the 8 axon-tunneled NeuronCores via PJRT; a
# JAX_PLATFORMS=cpu pin (sometimes used for pure-host reference runs) would
# hide the devices, so clear it before anything imports jax.
if os.environ.get("JAX_PLATFORMS") == "cpu":
    os.environ.pop("JAX_PLATFORMS")

import numpy as np

import concourse.bacc as bacc
import concourse.mybir as mybir
import concourse.tile as tile
from concourse.bass import ds

B, L, D, H, KER, DOUT = 2, 2048, 512, 8, 3, 512
DD = KER * D // H  # 192 per head
PAIR_CH = 2 * DD  # 384 per head-pair
SCALE = float(1.0 / np.sqrt(np.float32(KER * DD)))  # 1/24
P = 128
N_CORES = 8
FP = mybir.dt.float32

# matmul compute dtype: float32r streams at full PE rate (fp32 is 4x slower).
# The BIR verifier requires fp32r matmul operands to be *produced* as
# float32r, so every matmul-feeding tensor/tile is declared float32r (same
# 4-byte layout; numpy side is plain float32).
MF = {
    "float32r": mybir.dt.float32r,
    "float32": mybir.dt.float32,
}[os.environ.get("KERNEL_MM_DT", "float32r")]


def _mm(ap):
    return ap


def _build_body(tc, aps):
    nc = tc.nc
    xq_t, xk_t, xv_t = aps["xq_t"], aps["xk_t"], aps["xv_t"]
    wq_e, wk_e, wv_e = aps["wq_e"], aps["wk_e"], aps["wv_e"]
    bq_e, bk_e, bv_e = aps["bq_e"], aps["bk_e"], aps["bv_e"]
    wo_e, shifts = aps["wo_e"], aps["shifts"]
    attn_pair, out_part = aps["attn_pair"], aps["out_part"]

    AF = mybir.ActivationFunctionType
    ALU = mybir.AluOpType
    _B = lambda n, d: int(os.environ.get(n, d))

    with (
        # PSUM: lg+lt get dedicated banks; proj psum scope closes below and
        # its 4 banks are then reused by ctx/out psums (their first use is
        # after the projections drain anyway)
        tc.tile_pool(name="lpsum", bufs=_B("LG_BUFS", 2), space="PSUM") as lpsum,
        tc.tile_pool(name="ltpsum", bufs=_B("LT_BUFS", 2), space="PSUM") as ltpsum,
        tc.tile_pool(name="singles", bufs=1) as singles,
        tc.tile_pool(name="persist", bufs=1) as persist,
        tc.tile_pool(name="xT", bufs=_B("XT_BUFS", 3)) as xpool,
        tc.tile_pool(name="vtmp", bufs=_B("VTMP_BUFS", 3)) as vtmp,
        tc.tile_pool(name="attn", bufs=_B("ATTN_BUFS", 3)) as apool,
        tc.tile_pool(name="expT", bufs=_B("EXPT_BUFS", 4)) as epool,
        tc.tile_pool(name="small", bufs=_B("SMALL_BUFS", 4)) as spool,
        tc.tile_pool(name="out", bufs=_B("OUT_BUFS", 4)) as opool,
        tc.tile_pool(name="dram", bufs=1, space="DRAM") as drampool,
    ):
        # ---- persistent activation tiles (per-chunk for fine-grained deps) ----
        qpT = [
            persist.tile([P, 2052], MF, tag=f"qpT{c}", name=f"qpT{c}") for c in range(3)
        ]
        kpT = [
            persist.tile([P, 2052], MF, tag=f"kpT{c}", name=f"kpT{c}") for c in range(3)
        ]
        vp = [
            persist.tile([P, 16, DD], MF, tag=f"vp{h}", name=f"vp{h}") for h in range(2)
        ]  # [k, kc, chan] per head
        ctxB = [
            persist.tile([P, 4, 512], MF, tag=f"ctxB{h}", name=f"ctxB{h}")
            for h in range(2)
        ]
        ctxA = [
            persist.tile([64, 4, 512], MF, tag=f"ctxA{h}", name=f"ctxA{h}")
            for h in range(2)
        ]
        r1_all = persist.tile([P, 2, 16], FP, tag="r1")  # softmax reciprocals

        # ---- load constants / weights (v-path tensors first: they gate PE) ----
        wv_sb = singles.tile([P, 4, PAIR_CH], MF, tag="wv")
        nc.sync.dma_start(wv_sb, wv_e.rearrange("(o p) m -> p o m", p=P))
        xv_tiles = []
        for tg in range(4):
            t = xpool.tile([P, 4, 512], MF, tag="xT", name=f"xv{tg}")
            nc.sync.dma_start(
                t,
                xv_t.rearrange("(o p) t -> p o t", p=P)[
                    :, :, tg * 512 : (tg + 1) * 512
                ],
            )
            xv_tiles.append(t)
            if tg >= _B("XT_BUFS", 3) - 1:
                break  # remaining tgs loaded in the loop below

        wq_sb = singles.tile([P, 4, PAIR_CH], MF, tag="wq")
        wk_sb = singles.tile([P, 4, PAIR_CH], MF, tag="wk")
        nc.sync.dma_start(wq_sb, wq_e.rearrange("(o p) m -> p o m", p=P))
        nc.sync.dma_start(wk_sb, wk_e.rearrange("(o p) m -> p o m", p=P))
        wo_sb = singles.tile([P, 4, DOUT], MF, tag="wo")
        nc.sync.dma_start(wo_sb, wo_e.rearrange("(o p) m -> p o m", p=P))
        bq_sb = singles.tile([P, 3], FP, tag="bq")
        bk_sb = singles.tile([P, 3], FP, tag="bk")
        nc.sync.dma_start(bq_sb, bq_e.rearrange("(o p) -> p o", p=P))
        nc.sync.dma_start(bk_sb, bk_e.rearrange("(o p) -> p o", p=P))
        bv_sb = singles.tile([P, PAIR_CH], FP, tag="bv")
        nc.gpsimd.dma_start(
            bv_sb, bv_e.rearrange("(o c) -> o c", o=1).to_broadcast((P, PAIR_CH))
        )
        sh_sb = singles.tile([1, 16], mybir.dt.int32, tag="sh")
        nc.sync.dma_start(sh_sb, shifts)

        off_qp = [
            nc.values_load(
                sh_sb[0:1, i : i + 1],
                min_val=512 * (i % 4),
                max_val=512 * (i % 4) + 2,
                skip_runtime_bounds_check=True,
            )
            for i in range(12)
        ]
        off_vp = [
            nc.values_load(
                sh_sb[0:1, 12 + i : 13 + i],
                min_val=0,
                max_val=2,
                skip_runtime_bounds_check=True,
            )
            for i in range(3)
        ]

        for c in range(3):
            nc.vector.memset(qpT[c][:, 0:2].bitcast(FP), 0.0)
            nc.vector.memset(kpT[c][:, 0:2].bitcast(FP), 0.0)

        scr = drampool.tile([2 + L, PAIR_CH], MF)
        z2 = singles.tile([2, PAIR_CH], MF, tag="z2")
        nc.vector.memset(z2.bitcast(FP), 0.0)
        nc.sync.dma_start(scr[0:2, :], z2)

        # ---- projections ----
        with tc.tile_pool(name="ppsum", bufs=_B("PP_BUFS", 4), space="PSUM") as ppsum:
            # v path: xwv[t, ch] streamed to DRAM scratch (rows offset by 2)
            for tg in range(4):
                if tg < len(xv_tiles):
                    xv_tile = xv_tiles[tg]
                else:
                    xv_tile = xpool.tile([P, 4, 512], MF, tag="xT", name=f"xv{tg}")
                    nc.sync.dma_start(
                        xv_tile,
                        xv_t.rearrange("(o p) t -> p o t", p=P)[
                            :, :, tg * 512 : (tg + 1) * 512
                        ],
                    )
                for t4 in range(4):
                    ps = ppsum.tile([P, 512], FP, tag="pp", name="ppv")[:, :PAIR_CH]
                    for cc in range(4):
                        nc.tensor.matmul(
                            ps,
                            lhsT=_mm(xv_tile[:, cc, t4 * 128 : (t4 + 1) * 128]),
                            rhs=_mm(wv_sb[:, cc, :]),
                            start=(cc == 0),
                            stop=(cc == 3),
                        )
                    xw = vtmp.tile([P, PAIR_CH], MF, tag="xwv")
                    nc.vector.tensor_tensor(xw, ps, bv_sb, ALU.add)
                    t0 = 2 + tg * 512 + t4 * 128
                    nc.sync.dma_start(scr[t0 : t0 + 128, :], xw)

            # vp gather: [k, chan] per head with per-chunk time shift
            vspec = [
                (0, 0, 0, 128, 0),
                (0, 128, 128, 64, 1),
                (1, 0, 192, 64, 1),
                (1, 64, 256, 128, 2),
            ]
            for hh, d0, s0, w, ci in vspec:
                src = scr[ds(off_vp[ci], L), s0 : s0 + w].rearrange(
                    "(o p) c -> p o c", p=P
                )
                nc.sync.dma_start(vp[hh][:, :, d0 : d0 + w], src)

            # q/k paths: xwT[chan, t] written at dynamic column offset
            for x_ap, w_sb, b_sb, dst, use_act in [
                (xq_t, wq_sb, bq_sb, qpT, True),
                (xk_t, wk_sb, bk_sb, kpT, False),
            ]:
                for tg in range(4):
                    xt = xpool.tile([P, 4, 512], MF, tag="xT")
                    nc.sync.dma_start(
                        xt,
                        x_ap.rearrange("(o p) t -> p o t", p=P)[
                            :, :, tg * 512 : (tg + 1) * 512
                        ],
                    )
                    for ck in range(3):
                        ps = ppsum.tile([P, 512], FP, tag="pp", name="ppqk")
                        for cc in range(4):
                            nc.tensor.matmul(
                                ps,
                                lhsT=_mm(w_sb[:, cc, ck * 128 : (ck + 1) * 128]),
                                rhs=_mm(xt[:, cc, :]),
                                start=(cc == 0),
                                stop=(cc == 3),
                            )
                        dstap = dst[ck][:, ds(off_qp[ck * 4 + tg], 512)]
                        if use_act:
                            nc.scalar.activation(
                                dstap, ps, AF.Identity, bias=b_sb[:, ck : ck + 1]
                            )
                        else:
                            nc.vector.tensor_scalar_add(dstap, ps, b_sb[:, ck : ck + 1])

        # ---- attention + per-qg output projection ----
        with (
            tc.tile_pool(name="cpsum", bufs=_B("CTX_BUFS", 1), space="PSUM") as cpsum,
            tc.tile_pool(name="opsum", bufs=_B("OP_BUFS", 2), space="PSUM") as opsum,
        ):
            KS_H = [
                [(0, 0, 128), (1, 0, 64)],
                [(1, 64, 64), (2, 0, 128)],
            ]
            for qg in range(4):
                for h in range(2):
                    KS = KS_H[h]
                    # [q,k] logits -> exp (+row sums) -> normalize -> attn out
                    for q4 in range(4) if not os.environ.get("SKIP_QK") else []:
                        qb = qg * 4 + q4
                        att = apool.tile([P, L], FP, tag="attn")
                        sums = spool.tile([P, 4], FP, tag="sums")
                        for kt in range(4):
                            ps = lpsum.tile([P, 512], FP, tag="lg")
                            for i, (c, p0, pc) in enumerate(KS):
                                nc.tensor.matmul(
                                    ps,
                                    lhsT=_mm(
                                        qpT[c][p0 : p0 + pc, qb * 128 : (qb + 1) * 128]
                                    ),
                                    rhs=_mm(
                                        kpT[c][p0 : p0 + pc, kt * 512 : (kt + 1) * 512]
                                    ),
                                    start=(i == 0),
                                    stop=(i == len(KS) - 1),
                                )
                            nc.scalar.activation(
                                att[:, kt * 512 : (kt + 1) * 512],
                                ps,
                                AF.Exp,
                                scale=SCALE,
                                accum_out=sums[:, kt : kt + 1],
                            )
                        s1 = spool.tile([P, 1], FP, tag="s1")
                        nc.vector.reduce_sum(s1, sums, axis=mybir.AxisListType.X)
                        nc.vector.reciprocal(r1_all[:, h, qb : qb + 1], s1)
                        nc.vector.tensor_scalar_mul(att, att, r1_all[:, h, qb : qb + 1])
                        nc.sync.dma_start(
                            attn_pair[h, qb * 128 : (qb + 1) * 128, :], att
                        )

                    if os.environ.get("SKIP_CTX"):
                        continue
                    # [k,q] logits -> exp -> ctx accumulation (unnormalized)
                    psB = cpsum.tile([P, 512], FP, tag="cB")
                    psA = cpsum.tile([64, 512], FP, tag="cA")
                    for kc in range(16):
                        pst = ltpsum.tile([P, 512], FP, tag="lt")
                        for i, (c, p0, pc) in enumerate(KS):
                            nc.tensor.matmul(
                                pst,
                                lhsT=_mm(
                                    kpT[c][p0 : p0 + pc, kc * 128 : (kc + 1) * 128]
                                ),
                                rhs=_mm(
                                    qpT[c][p0 : p0 + pc, qg * 512 : (qg + 1) * 512]
                                ),
                                start=(i == 0),
                                stop=(i == len(KS) - 1),
                            )
                        et = epool.tile([P, 512], MF, tag="expT")
                        nc.scalar.activation(et, pst, AF.Exp, scale=SCALE)
                        nc.tensor.matmul(
                            psB,
                            lhsT=_mm(vp[h][:, kc, 0:128]),
                            rhs=_mm(et),
                            start=(kc == 0),
                            stop=(kc == 15),
                        )
                        nc.tensor.matmul(
                            psA,
                            lhsT=_mm(vp[h][:, kc, 128:DD]),
                            rhs=_mm(et),
                            start=(kc == 0),
                            stop=(kc == 15),
                        )
                    nc.vector.tensor_copy(ctxB[h][:, qg, :], psB)
                    nc.vector.tensor_copy(ctxA[h][:, qg, :], psA)

                if (
                    os.environ.get("SKIP_OUT")
                    or os.environ.get("SKIP_CTX")
                    or os.environ.get("SKIP_QK")
                ):
                    continue
                # output projection for this qg (softmax recip folded in)
                for t4 in range(4):
                    qb = qg * 4 + t4
                    ot = opool.tile([P, DOUT], FP, tag="ot")
                    for h in range(2):
                        ps = opsum.tile([P, DOUT], FP, tag="op")
                        nc.tensor.matmul(
                            ps,
                            lhsT=_mm(ctxB[h][:, qg, t4 * 128 : (t4 + 1) * 128]),
                            rhs=_mm(wo_sb[:, 2 * h, :]),
                            start=True,
                            stop=False,
                        )
                        nc.tensor.matmul(
                            ps,
                            lhsT=_mm(ctxA[h][:, qg, t4 * 128 : (t4 + 1) * 128]),
                            rhs=_mm(wo_sb[0:64, 2 * h + 1, :]),
                            start=False,
                            stop=True,
                        )
                        if h == 0:
                            nc.vector.tensor_scalar_mul(
                                ot, ps, r1_all[:, 0, qb : qb + 1]
                            )
                        else:
                            nc.vector.scalar_tensor_tensor(
                                ot,
                                ps,
                                r1_all[:, 1, qb : qb + 1],
                                ot,
                                op0=ALU.mult,
                                op1=ALU.add,
                            )
                    nc.sync.dma_start(out_part[qb * 128 : (qb + 1) * 128, :], ot)


def build_nc():
    nc = bacc.Bacc("TRN2", target_bir_lowering=False, debug=False)
    aps = {}

    def inp(name, shape, dtype=FP):
        aps[name] = nc.dram_tensor(name, shape, dtype, kind="ExternalInput").ap()

    def outp(name, shape, dtype=FP):
        aps[name] = nc.dram_tensor(name, shape, dtype, kind="ExternalOutput").ap()

    inp("xq_t", [D, L], MF)
    inp("xk_t", [D, L], MF)
    inp("xv_t", [D, L], MF)
    inp("wq_e", [D, PAIR_CH], MF)
    inp("wk_e", [D, PAIR_CH], MF)
    inp("wv_e", [D, PAIR_CH], MF)
    inp("bq_e", [PAIR_CH])
    inp("bk_e", [PAIR_CH])
    inp("bv_e", [PAIR_CH])
    inp("wo_e", [D, DOUT], MF)
    inp("shifts", [1, 16], mybir.dt.int32)
    outp("attn_pair", [2, L, L])
    outp("out_part", [L, DOUT])

    with tile.TileContext(nc) as tc:
        _build_body(tc, aps)
    nc.compile()
    return nc


def make_in_maps(inputs):
    """Shard/transform full inputs into the 8 per-core input maps."""
    q = np.asarray(inputs["q"], np.float32)
    k = np.asarray(inputs["k"], np.float32)
    v = np.asarray(inputs["v"], np.float32)
    wq = np.asarray(inputs["wq"], np.float32)
    wk = np.asarray(inputs["wk"], np.float32)
    wv = np.asarray(inputs["wv"], np.float32)
    bq = np.asarray(inputs["bq"], np.float32)
    bk = np.asarray(inputs["bk"], np.float32)
    bv = np.asarray(inputs["bv"], np.float32)
    wo = np.asarray(inputs["wo"], np.float32)

    xT = {
        (n, b): np.ascontiguousarray(x[b].T)
        for n, x in [("xq_t", q), ("xk_t", k), ("xv_t", v)]
        for b in range(B)
    }

    in_maps = []
    for c in range(N_CORES):
        b, p = divmod(c, 4)
        base = PAIR_CH * p
        cols = np.arange(base, base + PAIR_CH) % D  # x@W column per pair channel
        # time shift per 128-chunk of the pair's channels (uniform within chunk)
        s = [KER - 1 - (base + 128 * ck) // D for ck in range(3)]

        wo_e = np.zeros((D, DOUT), np.float32)
        wo_e[0:128] = wo[base : base + 128]  # head0 B chunk (K=128)
        wo_e[128:192] = wo[base + 128 : base + 192]  # head0 A chunk (K=64)
        wo_e[256:384] = wo[base + 192 : base + 320]  # head1 B chunk (K=128)
        wo_e[384:448] = wo[base + 320 : base + 384]  # head1 A chunk (K=64)

        sh = np.zeros((1, 16), np.int32)
        for ck in range(3):
            for tg in range(4):
                sh[0, ck * 4 + tg] = s[ck] + 512 * tg
            sh[0, 12 + ck] = 2 - s[ck]

        in_maps.append(
            {
                "xq_t": xT[("xq_t", b)],
                "xk_t": xT[("xk_t", b)],
                "xv_t": xT[("xv_t", b)],
                "wq_e": np.ascontiguousarray(wq[:, cols]),
                "wk_e": np.ascontiguousarray(wk[:, cols]),
                "wv_e": np.ascontiguousarray(wv[:, cols]),
                "bq_e": np.ascontiguousarray(bq[cols]),
                "bk_e": np.ascontiguousarray(bk[cols]),
                "bv_e": np.ascontiguousarray(bv[cols]),
                "wo_e": wo_e,
                "shifts": sh,
            }
        )
    return in_maps


_NC_CACHE = None


def _get_nc():
    global _NC_CACHE
    if _NC_CACHE is None:
        _NC_CACHE = build_nc()
    return _NC_CACHE


_RUNNER = None


def _get_runner():
    """Build (once) a cached 8-core PJRT executable for the Bass program.

    Mirrors concourse.bass2jax.run_bass_via_pjrt, but caches the jitted
    callable so repeated kernel() calls don't re-trace/re-compile, and
    creates the donated output buffers on-device (no 270MB zero upload).
    """
    global _RUNNER
    if _RUNNER is not None:
        return _RUNNER

    import jax
    import jax.numpy as jnp
    from jax.sharding import Mesh, PartitionSpec, NamedSharding
    from jax.experimental.shard_map import shard_map
    from concourse import bass2jax

    nc = _get_nc()
    bass2jax.install_neuronx_cc_hook()

    partition_name = nc.partition_id_tensor.name if nc.partition_id_tensor else None
    in_names, out_names, out_avals = [], [], []
    for alloc in nc.m.functions[0].allocations:
        if not isinstance(alloc, mybir.MemoryLocationSet):
            continue
        name = alloc.memorylocations[0].name
        if alloc.kind == "ExternalInput":
            if name != partition_name:
                in_names.append(name)
        elif alloc.kind == "ExternalOutput":
            out_names.append(name)
            out_avals.append(
                jax.core.ShapedArray(
                    tuple(alloc.tensor_shape), mybir.dt.np(alloc.dtype)
                )
            )
    n_params = len(in_names)
    n_outs = len(out_avals)
    all_in_names = tuple(
        in_names + out_names + ([partition_name] if partition_name else [])
    )

    def _body(*args):
        operands = list(args)
        if partition_name is not None:
            operands.append(bass2jax.partition_id_tensor())
        outs = bass2jax._bass_exec_p.bind(
            *operands,
            out_avals=tuple(out_avals),
            in_names=all_in_names,
            out_names=tuple(out_names),
            lowering_input_output_aliases=(),
            sim_require_finite=True,
            sim_require_nnan=True,
            nc=nc,
        )
        return tuple(outs)

    devices = jax.devices()[:N_CORES]
    assert len(devices) == N_CORES, f"need {N_CORES} devices, got {len(devices)}"
    mesh = Mesh(np.asarray(devices), ("core",))
    spec = PartitionSpec("core")
    sharding = NamedSharding(mesh, spec)
    donate = tuple(range(n_params, n_params + n_outs))
    sharded = jax.jit(
        shard_map(
            _body,
            mesh=mesh,
            in_specs=(spec,) * (n_params + n_outs),
            out_specs=(spec,) * n_outs,
            check_rep=False,
        ),
        donate_argnums=donate,
        keep_unused=True,
    )
    # on-device zero buffers for the donated outputs
    zero_shapes = [
        (N_CORES * a.shape[0], *a.shape[1:]) for a in out_avals
    ]
    make_zeros = jax.jit(
        lambda: tuple(jnp.zeros(s, np.float32) for s in zero_shapes),
        out_shardings=(sharding,) * n_outs,
    )

    from concurrent.futures import ThreadPoolExecutor

    # input duplication structure: x tensors depend only on batch (c//4),
    # everything else only on head-pair (c%4)
    X_NAMES = {"xq_t", "xk_t", "xv_t"}

    def runner(in_maps, timings=None):
        t0 = time.monotonic()
        # 1) tunnel-upload each unique buffer once
        unique = {}  # (name, key) -> device array on its "home" device
        for name in in_names:
            for c in range(N_CORES):
                key = c // 4 if name in X_NAMES else c % 4
                if (name, key) not in unique:
                    unique[(name, key)] = jax.device_put(
                        np.asarray(in_maps[c][name]), devices[c]
                    )
        jax.block_until_ready(list(unique.values()))
        # 2) replicate to remaining devices over D2D
        parts_by_name = {name: [None] * N_CORES for name in in_names}
        for (name, key), arr in unique.items():
            home = key * 4 if name in X_NAMES else key  # device index it lives on
            parts_by_name[name][home] = arr
        d2d = []
        for name in in_names:
            for c in range(N_CORES):
                if parts_by_name[name][c] is None:
                    key = c // 4 if name in X_NAMES else c % 4
                    home = key * 4 if name in X_NAMES else key
                    a = jax.device_put(parts_by_name[name][home], devices[c])
                    parts_by_name[name][c] = a
                    d2d.append(a)
        jax.block_until_ready(d2d)
        dev_in = []
        for name in in_names:
            parts = parts_by_name[name]
            shape = parts[0].shape
            dev_in.append(
                jax.make_array_from_single_device_arrays(
                    (N_CORES * shape[0], *shape[1:]), sharding, parts
                )
            )
        zeros = make_zeros()
        jax.block_until_ready(zeros)
        t1 = time.monotonic()
        out_arrs = sharded(*dev_in, *zeros)
        jax.block_until_ready(out_arrs)
        t2 = time.monotonic()
        # 3) fetch all shards concurrently, writing attn straight into place
        attn = np.empty((B, H, L, L), np.float32)
        out_parts = [None] * N_CORES
        name_idx = {n: i for i, n in enumerate(out_names)}
        attn_shards = list(out_arrs[name_idx["attn_pair"]].addressable_shards)
        out_shards = list(out_arrs[name_idx["out_part"]].addressable_shards)

        def fetch_attn(c):
            b, p = divmod(c, 4)
            np.copyto(attn[b, 2 * p : 2 * p + 2], np.asarray(attn_shards[c].data))

        def fetch_out(c):
            out_parts[c] = np.asarray(out_shards[c].data)

        with ThreadPoolExecutor(16) as ex:
            futs = [ex.submit(fetch_attn, c) for c in range(N_CORES)]
            futs += [ex.submit(fetch_out, c) for c in range(N_CORES)]
            for f in futs:
                f.result()
        t3 = time.monotonic()
        if timings is not None:
            timings.update(upload=t1 - t0, exec=t2 - t1, download=t3 - t2)
        return attn, out_parts

    _RUNNER = runner
    return runner


def run(inputs, timings=None, **_ignored):
    """Run on 8 cores; returns (out, attn, per-phase timings dict)."""
    t0 = time.monotonic()
    in_maps = make_in_maps(inputs)
    t1 = time.monotonic()
    runner = _get_runner()
    tm = {}
    attn, out_parts = runner(in_maps, timings=tm)
    t2 = time.monotonic()

    bo = np.asarray(inputs["bo"], np.float32)
    out = np.zeros((B, L, DOUT), np.float32)
    for c in range(N_CORES):
        out[c // 4] += out_parts[c]
    out += bo
    t3 = time.monotonic()
    tm.update(shard=t1 - t0, run=t2 - t1, assemble=t3 - t2)
    if timings is not None:
        timings.update(tm)
    return out, attn, tm


def kernel(**inputs):
    out, attn, _ = run(inputs)
    return out, attn



# revision 32
# speedup vs baseline: 1.2132x; 1.2132x over previous
"""Trainium2 Bass kernel for nn_MultiHeadAttention_64158221467955.

Sliding-window (KER=3) patch multi-head attention:
    qp/kp/vp = patch_heads(x @ W + b)  -> [B,H,L,192]
    attn = softmax(qp @ kp^T / 24)     -> [B,H,L,L]   (returned)
    out  = combine(attn @ vp) @ wo + bo -> [B,L,512]  (returned)

Sharding: 8 cores = (batch b in {0,1}) x (head-pair p in {0..3}).
Each core computes its pair's projections (weight columns host-sliced),
full attention for its 2 heads, and a partial output projection
(wo rows host-sliced); host sums partials and adds bo.

The patch structure means head-pair p's 384 channels are columns
(384p..384p+384)%512 of x@W, each 128-chunk read at a uniform time shift
s in {0,1,2}.  The shift varies per core, so it is fed as *data* (an
int32 offsets tensor) and applied with dynamic (register-driven) access
pattern offsets, keeping a single SPMD program.
"""

import os
import time

# This kernel executes on the 8 axon-tunneled NeuronCores via PJRT; a
# JAX_PLATFORMS=cpu pin (sometimes used for pure-host reference runs) would
# hide the devices, so clear it before anything imports jax.
if os.environ.get("JAX_PLATFORMS") == "cpu":
    os.environ.pop("JAX_PLATFORMS")

import numpy as np

import concourse.bacc as bacc
import concourse.mybir as mybir
import concourse.tile as tile
from concourse.bass import ds

B, L, D, H, KER, DOUT = 2, 2048, 512, 8, 3, 512
DD = KER * D // H  # 192 per head
PAIR_CH = 2 * DD  # 384 per head-pair
SCALE = float(1.0 / np.sqrt(np.float32(KER * DD)))  # 1/24
P = 128
N_CORES = 8
FP = mybir.dt.float32

# matmul compute dtype: float32r streams at full PE rate (fp32 is 4x slower).
# The BIR verifier requires fp32r matmul operands to be *produced* as
# float32r, so every matmul-feeding tensor/tile is declared float32r (same
# 4-byte layout; numpy side is plain float32).
MF = {
    "float32r": mybir.dt.float32r,
    "float32": mybir.dt.float32,
}[os.environ.get("KERNEL_MM_DT", "float32r")]


def _mm(ap):
    return ap


def _build_body(tc, aps):
    nc = tc.nc
    xq_t, xk_t, xv_t = aps["xq_t"], aps["xk_t"], aps["xv_t"]
    wq_e, wk_e, wv_e = aps["wq_e"], aps["wk_e"], aps["wv_e"]
    bq_e, bk_e, bv_e = aps["bq_e"], aps["bk_e"], aps["bv_e"]
    wo_e, shifts = aps["wo_e"], aps["shifts"]
    attn_pair, out_part = aps["attn_pair"], aps["out_part"]

    AF = mybir.ActivationFunctionType
    ALU = mybir.AluOpType
    _B = lambda n, d: int(os.environ.get(n, d))

    with (
        # PSUM: lg+lt get dedicated banks; proj psum scope closes below and
        # its 4 banks are then reused by ctx/out psums (their first use is
        # after the projections drain anyway)
        tc.tile_pool(name="lpsum", bufs=_B("LG_BUFS", 2), space="PSUM") as lpsum,
        tc.tile_pool(name="ltpsum", bufs=_B("LT_BUFS", 2), space="PSUM") as ltpsum,
        tc.tile_pool(name="singles", bufs=1) as singles,
        tc.tile_pool(name="persist", bufs=1) as persist,
        tc.tile_pool(name="xT", bufs=_B("XT_BUFS", 3)) as xpool,
        tc.tile_pool(name="vtmp", bufs=_B("VTMP_BUFS", 3)) as vtmp,
        tc.tile_pool(name="attn", bufs=_B("ATTN_BUFS", 3)) as apool,
        tc.tile_pool(name="expT", bufs=_B("EXPT_BUFS", 4)) as epool,
        tc.tile_pool(name="small", bufs=_B("SMALL_BUFS", 4)) as spool,
        tc.tile_pool(name="out", bufs=_B("OUT_BUFS", 4)) as opool,
        tc.tile_pool(name="dram", bufs=1, space="DRAM") as drampool,
    ):
        # ---- persistent activation tiles (per-chunk for fine-grained deps) ----
        qpT = [
            persist.tile([P, 2052], MF, tag=f"qpT{c}", name=f"qpT{c}") for c in range(3)
        ]
        kpT = [
            persist.tile([P, 2052], MF, tag=f"kpT{c}", name=f"kpT{c}") for c in range(3)
        ]
        vp = [
            persist.tile([P, 16, DD], MF, tag=f"vp{h}", name=f"vp{h}") for h in range(2)
        ]  # [k, kc, chan] per head
        ctxB = [
            persist.tile([P, 4, 512], MF, tag=f"ctxB{h}", name=f"ctxB{h}")
            for h in range(2)
        ]
        ctxA = [
            persist.tile([64, 4, 512], MF, tag=f"ctxA{h}", name=f"ctxA{h}")
            for h in range(2)
        ]
        r1_all = persist.tile([P, 2, 16], FP, tag="r1")  # softmax reciprocals

        # ---- load constants / weights (v-path tensors first: they gate PE) ----
        wv_sb = singles.tile([P, 4, PAIR_CH], MF, tag="wv")
        nc.sync.dma_start(wv_sb, wv_e.rearrange("(o p) m -> p o m", p=P))
        xv_tiles = []
        for tg in range(4):
            t = xpool.tile([P, 4, 512], MF, tag="xT", name=f"xv{tg}")
            nc.sync.dma_start(
                t,
                xv_t.rearrange("(o p) t -> p o t", p=P)[
                    :, :, tg * 512 : (tg + 1) * 512
                ],
            )
            xv_tiles.append(t)
            if tg >= _B("XT_BUFS", 3) - 1:
                break  # remaining tgs loaded in the loop below

        wq_sb = singles.tile([P, 4, PAIR_CH], MF, tag="wq")
        wk_sb = singles.tile([P, 4, PAIR_CH], MF, tag="wk")
        nc.sync.dma_start(wq_sb, wq_e.rearrange("(o p) m -> p o m", p=P))
        nc.sync.dma_start(wk_sb, wk_e.rearrange("(o p) m -> p o m", p=P))
        wo_sb = singles.tile([P, 4, DOUT], MF, tag="wo")
        nc.sync.dma_start(wo_sb, wo_e.rearrange("(o p) m -> p o m", p=P))
        bq_sb = singles.tile([P, 3], FP, tag="bq")
        bk_sb = singles.tile([P, 3], FP, tag="bk")
        nc.sync.dma_start(bq_sb, bq_e.rearrange("(o p) -> p o", p=P))
        nc.sync.dma_start(bk_sb, bk_e.rearrange("(o p) -> p o", p=P))
        bv_sb = singles.tile([P, PAIR_CH], FP, tag="bv")
        nc.gpsimd.dma_start(
            bv_sb, bv_e.rearrange("(o c) -> o c", o=1).to_broadcast((P, PAIR_CH))
        )
        sh_sb = singles.tile([1, 16], mybir.dt.int32, tag="sh")
        nc.sync.dma_start(sh_sb, shifts)

        off_qp = [
            nc.values_load(
                sh_sb[0:1, i : i + 1],
                min_val=512 * (i % 4),
                max_val=512 * (i % 4) + 2,
                skip_runtime_bounds_check=True,
            )
            for i in range(12)
        ]
        off_vp = [
            nc.values_load(
                sh_sb[0:1, 12 + i : 13 + i],
                min_val=0,
                max_val=2,
                skip_runtime_bounds_check=True,
            )
            for i in range(3)
        ]

        for c in range(3):
            nc.vector.memset(qpT[c][:, 0:2].bitcast(FP), 0.0)
            nc.vector.memset(kpT[c][:, 0:2].bitcast(FP), 0.0)

        scr = drampool.tile([2 + L, PAIR_CH], MF)
        z2 = singles.tile([2, PAIR_CH], MF, tag="z2")
        nc.vector.memset(z2.bitcast(FP), 0.0)
        nc.sync.dma_start(scr[0:2, :], z2)

        # ---- projections ----
        with tc.tile_pool(name="ppsum", bufs=_B("PP_BUFS", 4), space="PSUM") as ppsum:
            # v path: xwv[t, ch] streamed to DRAM scratch (rows offset by 2)
            for tg in range(4):
                if tg < len(xv_tiles):
                    xv_tile = xv_tiles[tg]
                else:
                    xv_tile = xpool.tile([P, 4, 512], MF, tag="xT", name=f"xv{tg}")
                    nc.sync.dma_start(
                        xv_tile,
                        xv_t.rearrange("(o p) t -> p o t", p=P)[
                            :, :, tg * 512 : (tg + 1) * 512
                        ],
                    )
                for t4 in range(4):
                    ps = ppsum.tile([P, 512], FP, tag="pp", name="ppv")[:, :PAIR_CH]
                    for cc in range(4):
                        nc.tensor.matmul(
                            ps,
                            lhsT=_mm(xv_tile[:, cc, t4 * 128 : (t4 + 1) * 128]),
                            rhs=_mm(wv_sb[:, cc, :]),
                            start=(cc == 0),
                            stop=(cc == 3),
                        )
                    xw = vtmp.tile([P, PAIR_CH], MF, tag="xwv")
                    nc.vector.tensor_tensor(xw, ps, bv_sb, ALU.add)
                    t0 = 2 + tg * 512 + t4 * 128
                    nc.sync.dma_start(scr[t0 : t0 + 128, :], xw)

            # vp gather: [k, chan] per head with per-chunk time shift
            vspec = [
                (0, 0, 0, 128, 0),
                (0, 128, 128, 64, 1),
                (1, 0, 192, 64, 1),
                (1, 64, 256, 128, 2),
            ]
            for hh, d0, s0, w, ci in vspec:
                src = scr[ds(off_vp[ci], L), s0 : s0 + w].rearrange(
                    "(o p) c -> p o c", p=P
                )
                nc.sync.dma_start(vp[hh][:, :, d0 : d0 + w], src)

            # q/k paths: xwT[chan, t] written at dynamic column offset
            for x_ap, w_sb, b_sb, dst, use_act in [
                (xq_t, wq_sb, bq_sb, qpT, False),
                (xk_t, wk_sb, bk_sb, kpT, False),
            ]:
                for tg in range(4):
                    xt = xpool.tile([P, 4, 512], MF, tag="xT")
                    nc.sync.dma_start(
                        xt,
                        x_ap.rearrange("(o p) t -> p o t", p=P)[
                            :, :, tg * 512 : (tg + 1) * 512
                        ],
                    )
                    for ck in range(3):
                        ps = ppsum.tile([P, 512], FP, tag="pp", name="ppqk")
                        for cc in range(4):
                            nc.tensor.matmul(
                                ps,
                                lhsT=_mm(w_sb[:, cc, ck * 128 : (ck + 1) * 128]),
                                rhs=_mm(xt[:, cc, :]),
                                start=(cc == 0),
                                stop=(cc == 3),
                            )
                        dstap = dst[ck][:, ds(off_qp[ck * 4 + tg], 512)]
                        if use_act:
                            nc.scalar.activation(
                                dstap, ps, AF.Identity, bias=b_sb[:, ck : ck + 1]
                            )
                        else:
                            nc.vector.tensor_scalar_add(dstap, ps, b_sb[:, ck : ck + 1])

        # ---- attention + per-qg output projection ----
        with (
            tc.tile_pool(name="cpsum", bufs=_B("CTX_BUFS", 1), space="PSUM") as cpsum,
            tc.tile_pool(name="opsum", bufs=_B("OP_BUFS", 2), space="PSUM") as opsum,
        ):
            KS_H = [
                [(0, 0, 128), (1, 0, 64)],
                [(1, 64, 64), (2, 0, 128)],
            ]
            for qg in range(4):
                for h in range(2):
                    KS = KS_H[h]
                    # [q,k] logits -> exp (+row sums) -> normalize -> attn out
                    for q4 in range(4) if not os.environ.get("SKIP_QK") else []:
                        qb = qg * 4 + q4
                        att = apool.tile([P, L], FP, tag="attn")
                        sums = spool.tile([P, 4], FP, tag="sums")
                        for kt in range(4):
                            ps = lpsum.tile([P, 512], FP, tag="lg")
                            for i, (c, p0, pc) in enumerate(KS):
                                nc.tensor.matmul(
                                    ps,
                                    lhsT=_mm(
                                        qpT[c][p0 : p0 + pc, qb * 128 : (qb + 1) * 128]
                                    ),
                                    rhs=_mm(
                                        kpT[c][p0 : p0 + pc, kt * 512 : (kt + 1) * 512]
                                    ),
                                    start=(i == 0),
                                    stop=(i == len(KS) - 1),
                                )
                            nc.scalar.activation(
                                att[:, kt * 512 : (kt + 1) * 512],
                                ps,
                                AF.Exp,
                                scale=SCALE,
                                accum_out=sums[:, kt : kt + 1],
                            )
                        s1 = spool.tile([P, 1], FP, tag="s1")
                        nc.vector.reduce_sum(s1, sums, axis=mybir.AxisListType.X)
                        nc.vector.reciprocal(r1_all[:, h, qb : qb + 1], s1)
                        nc.vector.tensor_scalar_mul(att, att, r1_all[:, h, qb : qb + 1])
                        nc.sync.dma_start(
                            attn_pair[h, qb * 128 : (qb + 1) * 128, :], att
                        )

                    if os.environ.get("SKIP_CTX"):
                        continue
                    # [k,q] logits -> exp -> ctx accumulation (unnormalized)
                    psB = cpsum.tile([P, 512], FP, tag="cB")
                    psA = cpsum.tile([64, 512], FP, tag="cA")
                    for kc in range(16):
                        pst = ltpsum.tile([P, 512], FP, tag="lt")
                        for i, (c, p0, pc) in enumerate(KS):
                            nc.tensor.matmul(
                                pst,
                                lhsT=_mm(
                                    kpT[c][p0 : p0 + pc, kc * 128 : (kc + 1) * 128]
                                ),
                                rhs=_mm(
                                    qpT[c][p0 : p0 + pc, qg * 512 : (qg + 1) * 512]
                                ),
                                start=(i == 0),
                                stop=(i == len(KS) - 1),
                            )
                        et = epool.tile([P, 512], MF, tag="expT")
                        nc.scalar.activation(et, pst, AF.Exp, scale=SCALE)
                        nc.tensor.matmul(
                            psB,
                            lhsT=_mm(vp[h][:, kc, 0:128]),
                            rhs=_mm(et),
                            start=(kc == 0),
                            stop=(kc == 15),
                        )
                        nc.tensor.matmul(
                            psA,
                            lhsT=_mm(vp[h][:, kc, 128:DD]),
                            rhs=_mm(et),
                            start=(kc == 0),
                            stop=(kc == 15),
                        )
                    nc.vector.tensor_copy(ctxB[h][:, qg, :], psB)
                    nc.vector.tensor_copy(ctxA[h][:, qg, :], psA)

                if (
                    os.environ.get("SKIP_OUT")
                    or os.environ.get("SKIP_CTX")
                    or os.environ.get("SKIP_QK")
                ):
                    continue
                # output projection for this qg (softmax recip folded in)
                for t4 in range(4):
                    qb = qg * 4 + t4
                    ot = opool.tile([P, DOUT], FP, tag="ot")
                    for h in range(2):
                        ps = opsum.tile([P, DOUT], FP, tag="op")
                        nc.tensor.matmul(
                            ps,
                            lhsT=_mm(ctxB[h][:, qg, t4 * 128 : (t4 + 1) * 128]),
                            rhs=_mm(wo_sb[:, 2 * h, :]),
                            start=True,
                            stop=False,
                        )
                        nc.tensor.matmul(
                            ps,
                            lhsT=_mm(ctxA[h][:, qg, t4 * 128 : (t4 + 1) * 128]),
                            rhs=_mm(wo_sb[0:64, 2 * h + 1, :]),
                            start=False,
                            stop=True,
                        )
                        if h == 0:
                            nc.vector.tensor_scalar_mul(
                                ot, ps, r1_all[:, 0, qb : qb + 1]
                            )
                        else:
                            nc.vector.scalar_tensor_tensor(
                                ot,
                                ps,
                                r1_all[:, 1, qb : qb + 1],
                                ot,
                                op0=ALU.mult,
                                op1=ALU.add,
                            )
                    nc.sync.dma_start(out_part[qb * 128 : (qb + 1) * 128, :], ot)


def build_nc():
    nc = bacc.Bacc("TRN2", target_bir_lowering=False, debug=False)
    aps = {}

    def inp(name, shape, dtype=FP):
        aps[name] = nc.dram_tensor(name, shape, dtype, kind="ExternalInput").ap()

    def outp(name, shape, dtype=FP):
        aps[name] = nc.dram_tensor(name, shape, dtype, kind="ExternalOutput").ap()

    inp("xq_t", [D, L], MF)
    inp("xk_t", [D, L], MF)
    inp("xv_t", [D, L], MF)
    inp("wq_e", [D, PAIR_CH], MF)
    inp("wk_e", [D, PAIR_CH], MF)
    inp("wv_e", [D, PAIR_CH], MF)
    inp("bq_e", [PAIR_CH])
    inp("bk_e", [PAIR_CH])
    inp("bv_e", [PAIR_CH])
    inp("wo_e", [D, DOUT], MF)
    inp("shifts", [1, 16], mybir.dt.int32)
    outp("attn_pair", [2, L, L])
    outp("out_part", [L, DOUT])

    with tile.TileContext(nc) as tc:
        _build_body(tc, aps)
    nc.compile()
    return nc


def make_in_maps(inputs):
    """Shard/transform full inputs into the 8 per-core input maps."""
    q = np.asarray(inputs["q"], np.float32)
    k = np.asarray(inputs["k"], np.float32)
    v = np.asarray(inputs["v"], np.float32)
    wq = np.asarray(inputs["wq"], np.float32)
    wk = np.asarray(inputs["wk"], np.float32)
    wv = np.asarray(inputs["wv"], np.float32)
    bq = np.asarray(inputs["bq"], np.float32)
    bk = np.asarray(inputs["bk"], np.float32)
    bv = np.asarray(inputs["bv"], np.float32)
    wo = np.asarray(inputs["wo"], np.float32)

    xT = {
        (n, b): np.ascontiguousarray(x[b].T)
        for n, x in [("xq_t", q), ("xk_t", k), ("xv_t", v)]
        for b in range(B)
    }

    in_maps = []
    for c in range(N_CORES):
        b, p = divmod(c, 4)
        base = PAIR_CH * p
        cols = np.arange(base, base + PAIR_CH) % D  # x@W column per pair channel
        # time shift per 128-chunk of the pair's channels (uniform within chunk)
        s = [KER - 1 - (base + 128 * ck) // D for ck in range(3)]

        wo_e = np.zeros((D, DOUT), np.float32)
        wo_e[0:128] = wo[base : base + 128]  # head0 B chunk (K=128)
        wo_e[128:192] = wo[base + 128 : base + 192]  # head0 A chunk (K=64)
        wo_e[256:384] = wo[base + 192 : base + 320]  # head1 B chunk (K=128)
        wo_e[384:448] = wo[base + 320 : base + 384]  # head1 A chunk (K=64)

        sh = np.zeros((1, 16), np.int32)
        for ck in range(3):
            for tg in range(4):
                sh[0, ck * 4 + tg] = s[ck] + 512 * tg
            sh[0, 12 + ck] = 2 - s[ck]

        in_maps.append(
            {
                "xq_t": xT[("xq_t", b)],
                "xk_t": xT[("xk_t", b)],
                "xv_t": xT[("xv_t", b)],
                "wq_e": np.ascontiguousarray(wq[:, cols]),
                "wk_e": np.ascontiguousarray(wk[:, cols]),
                "wv_e": np.ascontiguousarray(wv[:, cols]),
                "bq_e": np.ascontiguousarray(bq[cols]),
                "bk_e": np.ascontiguousarray(bk[cols]),
                "bv_e": np.ascontiguousarray(bv[cols]),
                "wo_e": wo_e,
                "shifts": sh,
            }
        )
    return in_maps


_NC_CACHE = None


def _get_nc():
    global _NC_CACHE
    if _NC_CACHE is None:
        _NC_CACHE = build_nc()
    return _NC_CACHE


_RUNNER = None


def _get_runner():
    """Build (once) a cached 8-core PJRT executable for the Bass program.

    Mirrors concourse.bass2jax.run_bass_via_pjrt, but caches the jitted
    callable so repeated kernel() calls don't re-trace/re-compile, and
    creates the donated output buffers on-device (no 270MB zero upload).
    """
    global _RUNNER
    if _RUNNER is not None:
        return _RUNNER

    import jax
    import jax.numpy as jnp
    from jax.sharding import Mesh, PartitionSpec, NamedSharding
    from jax.experimental.shard_map import shard_map
    from concourse import bass2jax

    nc = _get_nc()
    bass2jax.install_neuronx_cc_hook()

    partition_name = nc.partition_id_tensor.name if nc.partition_id_tensor else None
    in_names, out_names, out_avals = [], [], []
    for alloc in nc.m.functions[0].allocations:
        if not isinstance(alloc, mybir.MemoryLocationSet):
            continue
        name = alloc.memorylocations[0].name
        if alloc.kind == "ExternalInput":
            if name != partition_name:
                in_names.append(name)
        elif alloc.kind == "ExternalOutput":
            out_names.append(name)
            out_avals.append(
                jax.core.ShapedArray(
                    tuple(alloc.tensor_shape), mybir.dt.np(alloc.dtype)
                )
            )
    n_params = len(in_names)
    n_outs = len(out_avals)
    all_in_names = tuple(
        in_names + out_names + ([partition_name] if partition_name else [])
    )

    def _body(*args):
        operands = list(args)
        if partition_name is not None:
            operands.append(bass2jax.partition_id_tensor())
        outs = bass2jax._bass_exec_p.bind(
            *operands,
            out_avals=tuple(out_avals),
            in_names=all_in_names,
            out_names=tuple(out_names),
            lowering_input_output_aliases=(),
            sim_require_finite=True,
            sim_require_nnan=True,
            nc=nc,
        )
        return tuple(outs)

    devices = jax.devices()[:N_CORES]
    assert len(devices) == N_CORES, f"need {N_CORES} devices, got {len(devices)}"
    mesh = Mesh(np.asarray(devices), ("core",))
    spec = PartitionSpec("core")
    sharding = NamedSharding(mesh, spec)
    donate = tuple(range(n_params, n_params + n_outs))
    sharded = jax.jit(
        shard_map(
            _body,
            mesh=mesh,
            in_specs=(spec,) * (n_params + n_outs),
            out_specs=(spec,) * n_outs,
            check_rep=False,
        ),
        donate_argnums=donate,
        keep_unused=True,
    )
    # on-device zero buffers for the donated outputs
    zero_shapes = [
        (N_CORES * a.shape[0], *a.shape[1:]) for a in out_avals
    ]
    make_zeros = jax.jit(
        lambda: tuple(jnp.zeros(s, np.float32) for s in zero_shapes),
        out_shardings=(sharding,) * n_outs,
    )

    from concurrent.futures import ThreadPoolExecutor

    # input duplication structure: x tensors depend only on batch (c//4),
    # everything else only on head-pair (c%4)
    X_NAMES = {"xq_t", "xk_t", "xv_t"}

    def runner(in_maps, timings=None):
        t0 = time.monotonic()
        # 1) tunnel-upload each unique buffer once
        unique = {}  # (name, key) -> device array on its "home" device
        for name in in_names:
            for c in range(N_CORES):
                key = c // 4 if name in X_NAMES else c % 4
                if (name, key) not in unique:
                    unique[(name, key)] = jax.device_put(
                        np.asarray(in_maps[c][name]), devices[c]
                    )
        jax.block_until_ready(list(unique.values()))
        # 2) replicate to remaining devices over D2D
        parts_by_name = {name: [None] * N_CORES for name in in_names}
        for (name, key), arr in unique.items():
            home = key * 4 if name in X_NAMES else key  # device index it lives on
            parts_by_name[name][home] = arr
        d2d = []
        for name in in_names:
            for c in range(N_CORES):
                if parts_by_name[name][c] is None:
                    key = c // 4 if name in X_NAMES else c % 4
                    home = key * 4 if name in X_NAMES else key
                    a = jax.device_put(parts_by_name[name][home], devices[c])
                    parts_by_name[name][c] = a
                    d2d.append(a)
        jax.block_until_ready(d2d)
        dev_in = []
        for name in in_names:
            parts = parts_by_name[name]
            shape = parts[0].shape
            dev_in.append(
                jax.make_array_from_single_device_arrays(
                    (N_CORES * shape[0], *shape[1:]), sharding, parts
                )
            )
        zeros = make_zeros()
        jax.block_until_ready(zeros)
        t1 = time.monotonic()
        out_arrs = sharded(*dev_in, *zeros)
        jax.block_until_ready(out_arrs)
        t2 = time.monotonic()
        # 3) fetch all shards concurrently, writing attn straight into place
        attn = np.empty((B, H, L, L), np.float32)
        out_parts = [None] * N_CORES
        name_idx = {n: i for i, n in enumerate(out_names)}
        attn_shards = list(out_arrs[name_idx["attn_pair"]].addressable_shards)
        out_shards = list(out_arrs[name_idx["out_part"]].addressable_shards)

        def fetch_attn(c):
            b, p = divmod(c, 4)
            np.copyto(attn[b, 2 * p : 2 * p + 2], np.asarray(attn_shards[c].data))

        def fetch_out(c):
            out_parts[c] = np.asarray(out_shards[c].data)

        with ThreadPoolExecutor(16) as ex:
            futs = [ex.submit(fetch_attn, c) for c in range(N_CORES)]
            futs += [ex.submit(fetch_out, c) for c in range(N_CORES)]
            for f in futs:
                f.result()
        t3 = time.monotonic()
        if timings is not None:
            timings.update(upload=t1 - t0, exec=t2 - t1, download=t3 - t2)
        return attn, out_parts

    _RUNNER = runner
    return runner


def run(inputs, timings=None, **_ignored):
    """Run on 8 cores; returns (out, attn, per-phase timings dict)."""
    t0 = time.monotonic()
    in_maps = make_in_maps(inputs)
    t1 = time.monotonic()
    runner = _get_runner()
    tm = {}
    attn, out_parts = runner(in_maps, timings=tm)
    t2 = time.monotonic()

    bo = np.asarray(inputs["bo"], np.float32)
    out = np.zeros((B, L, DOUT), np.float32)
    for c in range(N_CORES):
        out[c // 4] += out_parts[c]
    out += bo
    t3 = time.monotonic()
    tm.update(shard=t1 - t0, run=t2 - t1, assemble=t3 - t2)
    if timings is not None:
        timings.update(tm)
    return out, attn, tm


def kernel(**inputs):
    out, attn, _ = run(inputs)
    return out, attn

